# revision 25
# baseline (speedup 1.0000x reference)
"""Trainium2 Bass kernel for nn_NodeNet (GNN message passing).

Strategy: data-parallel over graphs across 8 NeuronCores. Host transposes
inputs into [feature, row] layouts so every DMA is contiguous; all matmuls
bf16 (fp8 DoubleRow was tested and rejected: quantizing h1 to e4m3 alone
costs 2.6e-2 max-rel error, over the 2e-2 budget).

One fused launch per core runs a single unified software pipeline:
  - node tile t front (DMA x, l1, bias/relu) alongside node back(t-2)
    (l2, relu, datapoint-sum as 2x-mode tensor_tensor halving adds + reduce),
  - feature_enc emitted in batches of 4 node tiles (one N=64 matmul pair),
    delayed one iteration so the PE never waits on the DVE reduce,
  - the 4 edge tiles of node-chunk t-6, themselves 2-deep pipelined
    (front: attr DMA + fe broadcast from a x4-duplicated table + l1 + h1;
    back at t-2: l2 + h2; l3 for pairs of tiles is flushed one step late and
    packed into a single [128,512] PSUM bank via column tile_position, which
    halves the output-bias pointwise work and lets the two M=64 groups run
    concurrently in the PE array).
The next tile's attr DMA + fe broadcast are prefetched ahead of the current
pointwise ops so edge l1 never waits on the Vector queue. PSUM->SBUF pointwise
is split Scalar/Vector per the errata cost model ((172+FD)/1.2 vs
(120+FD)/0.96); output is written bf16 and upcast on the host. PSUM budget:
l1 a+b (2 banks) + l2 a+b double-buffered (4) + l3/feature_enc ring (2) = 8.

Structured fast path (edges grouped 128-per-graph, as produced by the
reference's setup_inputs) uses the fused program; a general fallback handles
arbitrary edge_index / batch via a node launch, host gather, edge launch.
Measured: 293.4us HW exec (baseline 425.6us), max rel err 7.5e-3.
"""

import os
import sys

import ml_dtypes
import numpy as np

BF16NP = ml_dtypes.bfloat16

if "/opt/trn_rl_repo" not in sys.path and os.path.isdir("/opt/trn_rl_repo"):
    sys.path.insert(0, "/opt/trn_rl_repo")

import concourse.bacc as bacc
import concourse.tile as tile
from concourse import mybir
from concourse.bass_utils import run_bass_kernel_spmd

G, ODE, NDATA, H, EA, EPG = 4096, 64, 32, 256, 64, 128
E = G * EPG
NCORES = 8
GC = G // NCORES           # graphs per core (512)
RC = GC * NDATA            # node-MLP rows per core (16384)
EC = GC * EPG              # edges per core (65536)
TN = 512                   # tile free size
NT = RC // TN              # node tiles (32)
ET = EC // TN              # edge tiles (128)
GPT = TN // EPG            # graphs per edge tile (4)

F32 = mybir.dt.float32
BF16 = mybir.dt.bfloat16
RELU = mybir.ActivationFunctionType.Relu
IDENT = mybir.ActivationFunctionType.Identity
ADD = mybir.AluOpType.add
MAX = mybir.AluOpType.max

_PROGRAMS = {}
last_results = None


def _install_trace_shim():
    """Optional: make trace=True work by injecting antenv.axon_hooks."""
    import types

    if "antenv.axon_hooks" in sys.modules:
        return
    try:
        mod = types.ModuleType("antenv.axon_hooks")
        mod._hook = None
        mod.set_axon_ntff_profile_hook = lambda h: setattr(mod, "_hook", h)
        mod.get_axon_ntff_profile_hook = lambda: mod._hook
        sys.modules["antenv.axon_hooks"] = mod
        import antenv

        antenv.axon_hooks = mod
        from trn_agent_boot.trn_boot import _ntff_profile_via_ctypes

        hook = _ntff_profile_via_ctypes("/opt/axon/libaxon_pjrt.so")
        if hook is not None:
            mod.set_axon_ntff_profile_hook(hook)
    except Exception:
        pass


def _declare_weights(nc):
    t = {}
    t["nw1"] = nc.dram_tensor("nw1", [128, H], BF16, kind="ExternalInput")
    t["nw2"] = nc.dram_tensor("nw2", [128, 2, H], BF16, kind="ExternalInput")
    t["nw3"] = nc.dram_tensor("nw3", [128, 2, ODE], BF16, kind="ExternalInput")
    t["nb1"] = nc.dram_tensor("nb1", [128, 2], F32, kind="ExternalInput")
    t["nb2"] = nc.dram_tensor("nb2", [128, 2], F32, kind="ExternalInput")
    t["nb3"] = nc.dram_tensor("nb3", [ODE, 1], F32, kind="ExternalInput")
    t["ew1"] = nc.dram_tensor("ew1", [128, H], BF16, kind="ExternalInput")
    t["ew2"] = nc.dram_tensor("ew2", [128, 2, H], BF16, kind="ExternalInput")
    t["ew3"] = nc.dram_tensor("ew3", [128, 2, ODE], BF16, kind="ExternalInput")
    t["eb1"] = nc.dram_tensor("eb1", [128, 2], F32, kind="ExternalInput")
    t["eb2"] = nc.dram_tensor("eb2", [128, 2], F32, kind="ExternalInput")
    t["eb3"] = nc.dram_tensor("eb3", [128, 1], F32, kind="ExternalInput")
    return t


def _load_weights(nc, consts, td, node: bool, edge: bool, sb=None):
    sb = {} if sb is None else sb
    names = []
    if node:
        names += ["nw1", "nw2", "nw3", "nb1", "nb2", "nb3"]
    if edge:
        names += ["ew1", "ew2", "ew3", "eb1", "eb2", "eb3"]
    for n in names:
        d = td[n]
        sb[n] = consts.tile(list(d.shape), d.dtype, tag=n, name=n)
        nc.sync.dma_start(sb[n], d[:])
    return sb


def _node_front(nc, pools, w, xT_d, t):
    consts, xin, hid, ps1, ps2, ps3 = pools
    xtp = xin.tile([128, TN], BF16, tag="nxt")
    nc.sync.dma_start(xtp, xT_d[:, t * TN:(t + 1) * TN])
    pa = ps1.tile([128, TN], F32, tag="l1a")
    pb = ps1.tile([128, TN], F32, tag="l1b")
    nc.tensor.matmul(pa, w["nw1"][:, 0:128], xtp, start=True, stop=True)
    nc.tensor.matmul(pb, w["nw1"][:, 128:256], xtp, start=True, stop=True)
    h1p = hid.tile([128, 2, TN], BF16, tag="nh1")
    nc.scalar.activation(h1p[:, 0], pa, RELU, bias=w["nb1"][:, 0:1])
    nc.vector.tensor_scalar(out=h1p[:, 1], in0=pb, scalar1=w["nb1"][:, 1:2],
                            scalar2=0.0, op0=ADD, op1=MAX)
    return (t, h1p)


def _node_back(nc, pools, w, hsum, prev2):
    consts, xin, hid, ps1, ps2, ps3 = pools
    tp, h1p_p = prev2
    pa = ps2.tile([128, TN], F32, tag="l2a")
    pb = ps2.tile([128, TN], F32, tag="l2b")
    for k in (0, 1):
        nc.tensor.matmul(pa, w["nw2"][:, k, 0:128], h1p_p[:, k],
                         start=(k == 0), stop=(k == 1))
    for k in (0, 1):
        nc.tensor.matmul(pb, w["nw2"][:, k, 128:256], h1p_p[:, k],
                         start=(k == 0), stop=(k == 1))
    h2p = hid.tile([128, 2, TN], BF16, tag="nh2")
    nc.scalar.activation(h2p[:, 0], pa, RELU, bias=w["nb2"][:, 0:1])
    nc.scalar.activation(h2p[:, 1], pb, RELU, bias=w["nb2"][:, 1:2])
    # sum over NDATA=32 datapoints of each of the 16 graphs:
    # two 2x-mode halving adds then one 1x reduce over the last 8
    with nc.allow_low_precision(reason="bf16 tree reduce feeds bf16 matmul"):
        v0 = h2p.rearrange("c k (g d) -> c k g d", d=NDATA)
        t1 = hid.tile([128, 2, 16, 16], BF16, tag="tr1")
        nc.vector.tensor_tensor(out=t1, in0=v0[:, :, :, 0:16], in1=v0[:, :, :, 16:32], op=ADD)
        t2 = hid.tile([128, 2, 16, 8], BF16, tag="tr2")
        nc.vector.tensor_tensor(out=t2, in0=t1[:, :, :, 0:8], in1=t1[:, :, :, 8:16], op=ADD)
        nc.vector.reduce_sum(out=hsum[:, :, tp], in_=t2,
                             axis=mybir.AxisListType.X)


def _fe_chunk(nc, pools, w, hsum, feT_sb, fe4T, c0, nch):
    """feature_enc for node tiles c0..c0+nch-1 (16 graphs each)."""
    consts, xin, hid, ps1, ps2, ps3 = pools
    ng = nch * 16
    ps_f = ps3.tile([128, TN], F32, tag="l3")
    for k in (0, 1):
        nc.tensor.matmul(ps_f[0:64, 0:ng], w["nw3"][:, k], hsum[:, k, c0:c0 + nch],
                         start=(k == 0), stop=(k == 1), tile_position=(0, 0))
    sl = slice(c0 * 16, c0 * 16 + ng)
    nc.scalar.activation(feT_sb[:, sl], ps_f[0:64, 0:ng], IDENT, bias=w["nb3"])
    with nc.allow_low_precision(reason="bf16 broadcast of bf16 data"):
        nc.vector.tensor_copy(
            out=fe4T[:, sl], in_=feT_sb[:, sl, None].to_broadcast([ODE, ng, 4]))


def _edge_prefetch(nc, pools, w, attrT_d, fe_src, st, t):
    consts, xin, hid, ps1, ps2, ps3 = pools
    e0 = t * TN
    g0 = t * GPT
    rtp = xin.tile([128, TN], BF16, tag="ert")
    nc.sync.dma_start(rtp[64:128], attrT_d[:, e0:e0 + TN])
    if fe_src[0] == "sbuf":
        fe4T = fe_src[1]
        # rtp[0:64] cols: e = glocal*128 + r*4 + f
        nc.vector.tensor_copy(
            out=rtp[0:64].rearrange("c (g r f) -> c g r f", g=GPT, f=4),
            in_=fe4T[:, g0:g0 + GPT, None, :].to_broadcast([ODE, GPT, EPG // 4, 4]),
        )
    else:
        nc.sync.dma_start(rtp[0:64], fe_src[1][:, e0:e0 + TN])
    st["rtpq"][t] = rtp


def _edge_step(nc, pools, w, attrT_d, outT_d, fe_src, st, t):
    """One pipelined edge step, 2 deep: front(t) = dma+bcast+l1+h1; back of
    st.prev2 (tile t-2) = l2+h2; l3/e3/out for the oldest pending pair once a
    third tile is pending, packed into one [128,TN] PSUM bank via column
    tile_position. st = dict(prev=..., prev2=..., pend=[...]). t=None -> drain."""
    consts, xin, hid, ps1, ps2, ps3 = pools
    prev = st["prev2"]
    pend = st["pend"]
    if True:
        cur = None
        if t is not None:
            if t not in st["rtpq"]:
                _edge_prefetch(nc, pools, w, attrT_d, fe_src, st, t)
            rtp = st["rtpq"].pop(t)
            pa = ps1.tile([128, TN], F32, tag="l1a")
            pb = ps1.tile([128, TN], F32, tag="l1b")
            nc.tensor.matmul(pa, w["ew1"][:, 0:128], rtp, start=True, stop=True)
            nc.tensor.matmul(pb, w["ew1"][:, 128:256], rtp, start=True, stop=True)
            # prefetch the next tile's inputs so its l1 never waits on the
            # broadcast; across a fe-batch boundary the columns aren't written
            # yet, so skip (that tile prefetches at its own step).
            if t + 1 < st["et"] and (t + 1) % 16 != 0:
                _edge_prefetch(nc, pools, w, attrT_d, fe_src, st, t + 1)
            h1p = hid.tile([128, 2, TN], BF16, tag="eh1")
            nc.scalar.activation(h1p[:, 0], pa, RELU, bias=w["eb1"][:, 0:1])
            nc.vector.tensor_scalar(out=h1p[:, 1], in0=pb, scalar1=w["eb1"][:, 1:2],
                                    scalar2=0.0, op0=ADD, op1=MAX)
            cur = (t, h1p)
        if prev is not None:
            tp, h1p_p = prev
            pa = ps2.tile([128, TN], F32, tag="l2a")
            pb = ps2.tile([128, TN], F32, tag="l2b")
            for k in (0, 1):
                nc.tensor.matmul(pa, w["ew2"][:, k, 0:128], h1p_p[:, k],
                                 start=(k == 0), stop=(k == 1))
            for k in (0, 1):
                nc.tensor.matmul(pb, w["ew2"][:, k, 128:256], h1p_p[:, k],
                                 start=(k == 0), stop=(k == 1))
            h2p = hid.tile([128, 2, TN], BF16, tag="eh2")
            nc.scalar.activation(h2p[:, 0], pa, RELU, bias=w["eb2"][:, 0:1])
            nc.vector.tensor_scalar(out=h2p[:, 1], in0=pb, scalar1=w["eb2"][:, 1:2],
                                    scalar2=0.0, op0=ADD, op1=MAX)
            pend.append((tp, h2p))
        nfl = 4 if len(pend) >= 5 else (len(pend) // 2 * 2 if t is None else 0)
        if nfl:
            # l3 for nfl pending tiles, col-tiled two-per-PSUM-bank: within a
            # bank the first tile -> partitions 0-63, second -> 64-127.
            # One k-outer burst across all tiles so the column groups run
            # concurrently and LDWEIGHTS stays covered.
            grp = pend[:nfl]
            banks = [ps3.tile([128, TN], F32, tag="l3", name=f"l3b{i}")
                     for i in range(nfl // 2)]
            for k in (0, 1):
                for j, (tj, h2j) in enumerate(grp):
                    nc.tensor.matmul(banks[j // 2][64 * (j % 2):64 * (j % 2) + 64],
                                     w["ew3"][:, k], h2j[:, k],
                                     start=(k == 0), stop=(k == 1),
                                     tile_position=(0, 64 * (j % 2)))
            for b in range(nfl // 2):
                otp = hid.tile([128, TN], BF16, tag="eot")
                with nc.allow_low_precision(reason="bf16 output, host upcasts"):
                    nc.scalar.activation(otp, banks[b], IDENT, bias=w["eb3"])
                e0a = grp[2 * b][0] * TN
                e0b = grp[2 * b + 1][0] * TN
                nc.sync.dma_start(outT_d[:, e0a:e0a + TN], otp[0:64])
                nc.sync.dma_start(outT_d[:, e0b:e0b + TN], otp[64:128])
            pend = pend[nfl:]
        st["prev2"] = st["prev"]
        st["prev"] = cur
        st["pend"] = pend


def _emit_edge_stage(nc, pools, w, attrT_d, outT_d, fe_src):
    st = {"prev": None, "prev2": None, "pend": [], "rtpq": {}, "et": ET}
    for t in range(ET):
        _edge_step(nc, pools, w, attrT_d, outT_d, fe_src, st, t)
    for _ in range(4):
        _edge_step(nc, pools, w, attrT_d, outT_d, fe_src, st, None)


def _build(mode):
    """mode: 'fused' (node+edge, fe on-chip), 'node', 'edge'."""
    nc = bacc.Bacc("TRN2", target_bir_lowering=False)
    td = _declare_weights(nc)
    if mode in ("fused", "node"):
        xT_d = nc.dram_tensor("xT", [128, RC], BF16, kind="ExternalInput")
    if mode in ("fused", "edge"):
        attrT_d = nc.dram_tensor("attrT", [64, EC], BF16, kind="ExternalInput")
        outT_d = nc.dram_tensor("outT", [64, EC], BF16, kind="ExternalOutput")
    if mode == "edge":
        feTg_d = nc.dram_tensor("feTg", [64, EC], BF16, kind="ExternalInput")
    if mode == "node":
        feT_out = nc.dram_tensor("feT", [ODE, GC], F32, kind="ExternalOutput")

    with tile.TileContext(nc) as tc:
        with (
            tc.tile_pool(name="consts", bufs=1) as consts,
            tc.tile_pool(name="xin", bufs=10) as xin,
            tc.tile_pool(name="hid", bufs=8) as hid,
            tc.tile_pool(name="ps1", bufs=1, space="PSUM") as ps1,
            tc.tile_pool(name="ps2", bufs=2, space="PSUM") as ps2,
            tc.tile_pool(name="ps3", bufs=2, space="PSUM") as ps3,
        ):
            pools = (consts, xin, hid, ps1, ps2, ps3)
            w = _load_weights(nc, consts, td,
                              node=mode in ("fused", "node"),
                              edge=mode == "edge")
            if mode == "fused":
                # unified pipeline: node tile t front || node back(t-2) ||
                # feature_enc batch of 4 chunks || the 4 edge tiles of chunk t-5.
                hsum = consts.tile([128, 2, NT, 16], BF16, tag="hsum")
                feT_sb = consts.tile([ODE, GC], BF16, tag="feT")
                fe4T = consts.tile([ODE, GC, 4], BF16, tag="fe4T")
                fe_src = ("sbuf", fe4T)
                st = {"prev": None, "prev2": None, "pend": [], "rtpq": {}, "et": ET}
                prev = prev2 = None
                pending_batch = None
                for t in range(NT + 6):
                    cur = _node_front(nc, pools, w, xT_d, t) if t < NT else None
                    if t == 0:
                        _load_weights(nc, consts, td, node=False, edge=True, sb=w)
                    if pending_batch is not None:
                        _fe_chunk(nc, pools, w, hsum, feT_sb, fe4T, pending_batch, 4)
                        pending_batch = None
                    if prev2 is not None:
                        _node_back(nc, pools, w, hsum, prev2)
                        if prev2[0] % 4 == 3:
                            pending_batch = prev2[0] - 3
                    c = t - 6
                    if c >= 0:
                        for e in range(4 * c, 4 * c + 4):
                            _edge_step(nc, pools, w, attrT_d, outT_d, fe_src, st, e)
                    prev2 = prev
                    prev = cur
                for _ in range(4):
                    _edge_step(nc, pools, w, attrT_d, outT_d, fe_src, st, None)
            elif mode == "node":
                hsum = consts.tile([128, 2, NT, 16], BF16, tag="hsum")
                prev = prev2 = None
                for t in range(NT + 2):
                    cur = _node_front(nc, pools, w, xT_d, t) if t < NT else None
                    if prev2 is not None:
                        _node_back(nc, pools, w, hsum, prev2)
                    prev2 = prev
                    prev = cur
                ps_f = ps3.tile([128, TN], F32, tag="l3")
                hs = hsum.rearrange("c k t g -> c k (t g)")
                for k in (0, 1):
                    nc.tensor.matmul(ps_f[0:64], w["nw3"][:, k], hs[:, k],
                                     start=(k == 0), stop=(k == 1), tile_position=(0, 0))
                feTf = consts.tile([ODE, GC], F32, tag="feTf")
                nc.scalar.activation(feTf, ps_f[0:64], IDENT, bias=w["nb3"])
                nc.sync.dma_start(feT_out[:], feTf)
            elif mode == "edge":
                _emit_edge_stage(nc, pools, w, attrT_d, outT_d, ("dram", feTg_d))
    nc.finalize()
    return nc


def _get_program(mode):
    if mode not in _PROGRAMS:
        _PROGRAMS[mode] = _build(mode)
    return _PROGRAMS[mode]


def _shared_weight_arrays(kw):
    f = np.float32
    c = np.ascontiguousarray
    b3 = np.asarray(kw["edge_b3"], dtype=f)
    return {
        "nw1": c(np.asarray(kw["node_w1"], dtype=f).astype(BF16NP)),
        "nw2": c(np.asarray(kw["node_w2"], dtype=f).reshape(2, 128, H).transpose(1, 0, 2).astype(BF16NP)),
        "nw3": c(np.asarray(kw["node_w3"], dtype=f).reshape(2, 128, ODE).transpose(1, 0, 2).astype(BF16NP)),
        "nb1": c(np.asarray(kw["node_b1"], dtype=f).reshape(2, 128).T),
        "nb2": c(np.asarray(kw["node_b2"], dtype=f).reshape(2, 128).T),
        "nb3": c(np.asarray(kw["node_b3"], dtype=f).reshape(ODE, 1)),
        "ew1": c(np.asarray(kw["edge_w1"], dtype=f).astype(BF16NP)),
        "ew2": c(np.asarray(kw["edge_w2"], dtype=f).reshape(2, 128, H).transpose(1, 0, 2).astype(BF16NP)),
        "ew3": c(np.asarray(kw["edge_w3"], dtype=f).reshape(2, 128, ODE).transpose(1, 0, 2).astype(BF16NP)),
        "eb1": c(np.asarray(kw["edge_b1"], dtype=f).reshape(2, 128).T),
        "eb2": c(np.asarray(kw["edge_b2"], dtype=f).reshape(2, 128).T),
        "eb3": c(np.concatenate([b3, b3]).reshape(128, 1)),
    }


def _x_transposed_per_core(x, c):
    xs = np.asarray(x, dtype=np.float32).reshape(G, ODE, 2, NDATA)[c * GC:(c + 1) * GC]
    return np.ascontiguousarray(xs.transpose(1, 2, 0, 3).reshape(128, RC).astype(BF16NP))


def kernel(x, edge_attr, node_w1, node_b1, node_w2, node_b2, node_w3, node_b3,
           edge_w1, edge_b1, edge_w2, edge_b2, edge_w3, edge_b3,
           edge_index, batch):
    global last_results
    kw = dict(x=x, node_w1=node_w1, node_b1=node_b1, node_w2=node_w2,
              node_b2=node_b2, node_w3=node_w3, node_b3=node_b3,
              edge_w1=edge_w1, edge_b1=edge_b1, edge_w2=edge_w2,
              edge_b2=edge_b2, edge_w3=edge_w3, edge_b3=edge_b3)
    trace = os.environ.get("KERNEL_TRACE", "") == "1"
    if trace:
        _install_trace_shim()

    edge_attr = np.asarray(edge_attr, dtype=np.float32)
    ei = np.asarray(edge_index)
    bt = np.asarray(batch)
    g_src = bt[ei[0]]
    g_dst = bt[ei[1]]
    same = g_src == g_dst
    structured = bool((g_src == np.repeat(np.arange(G), EPG)).all())

    shared = _shared_weight_arrays(kw)
    run_kwargs = dict(core_ids=list(range(NCORES)), trace=trace,
                      trace_cores=[0] if trace else None)

    if structured:
        nc = _get_program("fused")
        in_maps = []
        for c in range(NCORES):
            m = dict(shared)
            m["xT"] = _x_transposed_per_core(x, c)
            m["attrT"] = np.ascontiguousarray(edge_attr[c * EC:(c + 1) * EC].T.astype(BF16NP))
            in_maps.append(m)
        res = run_bass_kernel_spmd(nc, in_maps, **run_kwargs)
        last_results = res
        out = np.empty((E, EA), dtype=np.float32)
        for c in range(NCORES):
            out[c * EC:(c + 1) * EC] = res.results[c]["outT"].T.astype(np.float32)
    else:
        # general path: node stage -> host gather of feature_enc -> edge stage
        nc_node = _get_program("node")
        in_maps = []
        for c in range(NCORES):
            m = dict(shared)
            m["xT"] = _x_transposed_per_core(x, c)
            in_maps.append(m)
        res_n = run_bass_kernel_spmd(nc_node, in_maps, **run_kwargs)
        feT_full = np.concatenate([res_n.results[c]["feT"] for c in range(NCORES)],
                                  axis=1)          # [64, G]
        feTg = feT_full[:, g_src]                   # [64, E]
        nc_edge = _get_program("edge")
        in_maps = []
        for c in range(NCORES):
            m = dict(shared)
            m["attrT"] = np.ascontiguousarray(edge_attr[c * EC:(c + 1) * EC].T.astype(BF16NP))
            m["feTg"] = np.ascontiguousarray(feTg[:, c * EC:(c + 1) * EC].astype(BF16NP))
            in_maps.append(m)
        res = run_bass_kernel_spmd(nc_edge, in_maps, **run_kwargs)
        last_results = res
        out = np.empty((E, EA), dtype=np.float32)
        for c in range(NCORES):
            out[c * EC:(c + 1) * EC] = res.results[c]["outT"].T.astype(np.float32)
    if not same.all():
        out = np.where(same[:, None], out, edge_attr)
    return out


# revision 26
# speedup vs baseline: 1.0102x; 1.0102x over previous
"""Trainium2 Bass kernel for nn_NodeNet (GNN message passing).

Strategy: data-parallel over graphs across 8 NeuronCores. Host transposes
inputs into [feature, row] layouts so every DMA is contiguous; all matmuls
bf16 (fp8 DoubleRow was tested and rejected: quantizing h1 to e4m3 alone
costs 2.6e-2 max-rel error, over the 2e-2 budget).

One fused launch per core runs a single unified software pipeline:
  - node tile t front (DMA x, l1, bias/relu) alongside node back(t-2)
    (l2, relu, datapoint-sum as 2x-mode tensor_tensor halving adds + reduce),
  - feature_enc emitted in batches of 4 node tiles (one N=64 matmul pair),
    delayed one iteration so the PE never waits on the DVE reduce,
  - the 4 edge tiles of node-chunk t-6, themselves 2-deep pipelined
    (front: attr DMA + fe broadcast from a x4-duplicated table + l1 + h1;
    back at t-2: l2 + h2; l3 for pairs of tiles is flushed one step late and
    packed into a single [128,512] PSUM bank via column tile_position, which
    halves the output-bias pointwise work and lets the two M=64 groups run
    concurrently in the PE array).
The next tile's attr DMA + fe broadcast are prefetched ahead of the current
pointwise ops so edge l1 never waits on the Vector queue. PSUM->SBUF pointwise
is split Scalar/Vector per the errata cost model ((172+FD)/1.2 vs
(120+FD)/0.96); output is written bf16 and upcast on the host. PSUM budget:
l1 a+b (2 banks) + l2 a+b double-buffered (4) + l3/feature_enc ring (2) = 8.

Structured fast path (edges grouped 128-per-graph, as produced by the
reference's setup_inputs) uses the fused program; a general fallback handles
arbitrary edge_index / batch via a node launch, host gather, edge launch.
Measured: 293.4us HW exec (baseline 425.6us), max rel err 7.5e-3.
"""

import os
import sys

import ml_dtypes
import numpy as np

BF16NP = ml_dtypes.bfloat16

if "/opt/trn_rl_repo" not in sys.path and os.path.isdir("/opt/trn_rl_repo"):
    sys.path.insert(0, "/opt/trn_rl_repo")

import concourse.bacc as bacc
import concourse.tile as tile
from concourse import mybir
from concourse.bass_utils import run_bass_kernel_spmd

G, ODE, NDATA, H, EA, EPG = 4096, 64, 32, 256, 64, 128
E = G * EPG
NCORES = 8
GC = G // NCORES           # graphs per core (512)
RC = GC * NDATA            # node-MLP rows per core (16384)
EC = GC * EPG              # edges per core (65536)
TN = 512                   # tile free size
NT = RC // TN              # node tiles (32)
ET = EC // TN              # edge tiles (128)
GPT = TN // EPG            # graphs per edge tile (4)

F32 = mybir.dt.float32
BF16 = mybir.dt.bfloat16
RELU = mybir.ActivationFunctionType.Relu
IDENT = mybir.ActivationFunctionType.Identity
ADD = mybir.AluOpType.add
MAX = mybir.AluOpType.max

_PROGRAMS = {}
last_results = None


def _install_trace_shim():
    """Optional: make trace=True work by injecting antenv.axon_hooks."""
    import types

    if "antenv.axon_hooks" in sys.modules:
        return
    try:
        mod = types.ModuleType("antenv.axon_hooks")
        mod._hook = None
        mod.set_axon_ntff_profile_hook = lambda h: setattr(mod, "_hook", h)
        mod.get_axon_ntff_profile_hook = lambda: mod._hook
        sys.modules["antenv.axon_hooks"] = mod
        import antenv

        antenv.axon_hooks = mod
        from trn_agent_boot.trn_boot import _ntff_profile_via_ctypes

        hook = _ntff_profile_via_ctypes("/opt/axon/libaxon_pjrt.so")
        if hook is not None:
            mod.set_axon_ntff_profile_hook(hook)
    except Exception:
        pass


def _declare_weights(nc):
    t = {}
    t["nw1"] = nc.dram_tensor("nw1", [128, H], BF16, kind="ExternalInput")
    t["nw2"] = nc.dram_tensor("nw2", [128, 2, H], BF16, kind="ExternalInput")
    t["nw3"] = nc.dram_tensor("nw3", [128, 2, ODE], BF16, kind="ExternalInput")
    t["nb1"] = nc.dram_tensor("nb1", [128, 2], F32, kind="ExternalInput")
    t["nb2"] = nc.dram_tensor("nb2", [128, 2], F32, kind="ExternalInput")
    t["nb3"] = nc.dram_tensor("nb3", [ODE, 1], F32, kind="ExternalInput")
    t["ew1"] = nc.dram_tensor("ew1", [128, H], BF16, kind="ExternalInput")
    t["ew2"] = nc.dram_tensor("ew2", [128, 2, H], BF16, kind="ExternalInput")
    t["ew3"] = nc.dram_tensor("ew3", [128, 2, ODE], BF16, kind="ExternalInput")
    t["eb1"] = nc.dram_tensor("eb1", [128, 2], F32, kind="ExternalInput")
    t["eb2"] = nc.dram_tensor("eb2", [128, 2], F32, kind="ExternalInput")
    t["eb3"] = nc.dram_tensor("eb3", [128, 1], F32, kind="ExternalInput")
    return t


def _load_weights(nc, consts, td, node: bool, edge: bool, sb=None):
    sb = {} if sb is None else sb
    names = []
    if node:
        names += ["nw1", "nw2", "nw3", "nb1", "nb2", "nb3"]
    if edge:
        names += ["ew1", "ew2", "ew3", "eb1", "eb2", "eb3"]
    for n in names:
        d = td[n]
        sb[n] = consts.tile(list(d.shape), d.dtype, tag=n, name=n)
        nc.sync.dma_start(sb[n], d[:])
    return sb


def _node_front(nc, pools, w, xT_d, t):
    consts, xin, hid, ps1, ps2, ps3 = pools
    xtp = xin.tile([128, TN], BF16, tag="nxt")
    nc.sync.dma_start(xtp, xT_d[:, t * TN:(t + 1) * TN])
    pa = ps1.tile([128, TN], F32, tag="l1a")
    pb = ps1.tile([128, TN], F32, tag="l1b")
    nc.tensor.matmul(pa, w["nw1"][:, 0:128], xtp, start=True, stop=True)
    nc.tensor.matmul(pb, w["nw1"][:, 128:256], xtp, start=True, stop=True)
    h1p = hid.tile([128, 2, TN], BF16, tag="nh1")
    nc.scalar.activation(h1p[:, 0], pa, RELU, bias=w["nb1"][:, 0:1])
    nc.vector.tensor_scalar(out=h1p[:, 1], in0=pb, scalar1=w["nb1"][:, 1:2],
                            scalar2=0.0, op0=ADD, op1=MAX)
    return (t, h1p)


def _node_back(nc, pools, w, hsum, prev2):
    consts, xin, hid, ps1, ps2, ps3 = pools
    tp, h1p_p = prev2
    pa = ps2.tile([128, TN], F32, tag="l2a")
    pb = ps2.tile([128, TN], F32, tag="l2b")
    for k in (0, 1):
        nc.tensor.matmul(pa, w["nw2"][:, k, 0:128], h1p_p[:, k],
                         start=(k == 0), stop=(k == 1))
    for k in (0, 1):
        nc.tensor.matmul(pb, w["nw2"][:, k, 128:256], h1p_p[:, k],
                         start=(k == 0), stop=(k == 1))
    h2p = hid.tile([128, 2, TN], BF16, tag="nh2")
    nc.scalar.activation(h2p[:, 0], pa, RELU, bias=w["nb2"][:, 0:1])
    nc.scalar.activation(h2p[:, 1], pb, RELU, bias=w["nb2"][:, 1:2])
    # sum over NDATA=32 datapoints of each of the 16 graphs:
    # two 2x-mode halving adds then one 1x reduce over the last 8
    with nc.allow_low_precision(reason="bf16 tree reduce feeds bf16 matmul"):
        v0 = h2p.rearrange("c k (g d) -> c k g d", d=NDATA)
        t1 = hid.tile([128, 2, 16, 16], BF16, tag="tr1")
        nc.vector.tensor_tensor(out=t1, in0=v0[:, :, :, 0:16], in1=v0[:, :, :, 16:32], op=ADD)
        t2 = hid.tile([128, 2, 16, 8], BF16, tag="tr2")
        nc.vector.tensor_tensor(out=t2, in0=t1[:, :, :, 0:8], in1=t1[:, :, :, 8:16], op=ADD)
        nc.vector.reduce_sum(out=hsum[:, :, tp], in_=t2,
                             axis=mybir.AxisListType.X)


def _fe_chunk(nc, pools, w, hsum, feT_sb, fe4T, c0, nch):
    """feature_enc for node tiles c0..c0+nch-1 (16 graphs each)."""
    consts, xin, hid, ps1, ps2, ps3 = pools
    ng = nch * 16
    ps_f = ps3.tile([128, TN], F32, tag="l3")
    for k in (0, 1):
        nc.tensor.matmul(ps_f[0:64, 0:ng], w["nw3"][:, k], hsum[:, k, c0:c0 + nch],
                         start=(k == 0), stop=(k == 1), tile_position=(0, 0))
    sl = slice(c0 * 16, c0 * 16 + ng)
    nc.scalar.activation(feT_sb[:, sl], ps_f[0:64, 0:ng], IDENT, bias=w["nb3"])
    with nc.allow_low_precision(reason="bf16 broadcast of bf16 data"):
        nc.vector.tensor_copy(
            out=fe4T[:, sl], in_=feT_sb[:, sl, None].to_broadcast([ODE, ng, 4]))


def _edge_prefetch(nc, pools, w, attrT_d, fe_src, st, t):
    consts, xin, hid, ps1, ps2, ps3 = pools
    e0 = t * TN
    g0 = t * GPT
    rtp = xin.tile([128, TN], BF16, tag="ert")
    nc.sync.dma_start(rtp[64:128], attrT_d[:, e0:e0 + TN])
    if fe_src[0] == "sbuf":
        fe4T = fe_src[1]
        # rtp[0:64] cols: e = glocal*128 + r*4 + f
        nc.vector.tensor_copy(
            out=rtp[0:64].rearrange("c (g r f) -> c g r f", g=GPT, f=4),
            in_=fe4T[:, g0:g0 + GPT, None, :].to_broadcast([ODE, GPT, EPG // 4, 4]),
        )
    else:
        nc.sync.dma_start(rtp[0:64], fe_src[1][:, e0:e0 + TN])
    st["rtpq"][t] = rtp


def _edge_step(nc, pools, w, attrT_d, outT_d, fe_src, st, t):
    """One pipelined edge step, 2 deep: front(t) = dma+bcast+l1+h1; back of
    st.prev2 (tile t-2) = l2+h2; l3/e3/out for the oldest pending pair once a
    third tile is pending, packed into one [128,TN] PSUM bank via column
    tile_position. st = dict(prev=..., prev2=..., pend=[...]). t=None -> drain."""
    consts, xin, hid, ps1, ps2, ps3 = pools
    prev = st["prev2"]
    pend = st["pend"]
    if True:
        cur = None
        if t is not None:
            if t not in st["rtpq"]:
                _edge_prefetch(nc, pools, w, attrT_d, fe_src, st, t)
            rtp = st["rtpq"].pop(t)
            pa = ps1.tile([128, TN], F32, tag="l1a")
            pb = ps1.tile([128, TN], F32, tag="l1b")
            nc.tensor.matmul(pa, w["ew1"][:, 0:128], rtp, start=True, stop=True)
            nc.tensor.matmul(pb, w["ew1"][:, 128:256], rtp, start=True, stop=True)
            # prefetch the next tile's inputs so its l1 never waits on the
            # broadcast; across a fe-batch boundary the columns aren't written
            # yet, so skip (that tile prefetches at its own step).
            if t + 1 < st["et"] and (t + 1) % 16 != 0:
                _edge_prefetch(nc, pools, w, attrT_d, fe_src, st, t + 1)
            h1p = hid.tile([128, 2, TN], BF16, tag="eh1")
            nc.scalar.activation(h1p[:, 0], pa, RELU, bias=w["eb1"][:, 0:1])
            nc.vector.tensor_scalar(out=h1p[:, 1], in0=pb, scalar1=w["eb1"][:, 1:2],
                                    scalar2=0.0, op0=ADD, op1=MAX)
            cur = (t, h1p)
        if prev is not None:
            tp, h1p_p = prev
            pa = ps2.tile([128, TN], F32, tag="l2a")
            pb = ps2.tile([128, TN], F32, tag="l2b")
            for k in (0, 1):
                nc.tensor.matmul(pa, w["ew2"][:, k, 0:128], h1p_p[:, k],
                                 start=(k == 0), stop=(k == 1))
            for k in (0, 1):
                nc.tensor.matmul(pb, w["ew2"][:, k, 128:256], h1p_p[:, k],
                                 start=(k == 0), stop=(k == 1))
            h2p = hid.tile([128, 2, TN], BF16, tag="eh2")
            nc.scalar.activation(h2p[:, 0], pa, RELU, bias=w["eb2"][:, 0:1])
            nc.vector.tensor_scalar(out=h2p[:, 1], in0=pb, scalar1=w["eb2"][:, 1:2],
                                    scalar2=0.0, op0=ADD, op1=MAX)
            pend.append((tp, h2p))
        nfl = 2 if len(pend) >= 3 else (len(pend) // 2 * 2 if t is None else 0)
        if nfl:
            # l3 for nfl pending tiles, col-tiled two-per-PSUM-bank: within a
            # bank the first tile -> partitions 0-63, second -> 64-127.
            # One k-outer burst across all tiles so the column groups run
            # concurrently and LDWEIGHTS stays covered.
            grp = pend[:nfl]
            banks = [ps3.tile([128, TN], F32, tag="l3", name=f"l3b{i}")
                     for i in range(nfl // 2)]
            for k in (0, 1):
                for j, (tj, h2j) in enumerate(grp):
                    nc.tensor.matmul(banks[j // 2][64 * (j % 2):64 * (j % 2) + 64],
                                     w["ew3"][:, k], h2j[:, k],
                                     start=(k == 0), stop=(k == 1),
                                     tile_position=(0, 64 * (j % 2)))
            for b in range(nfl // 2):
                otp = hid.tile([128, TN], BF16, tag="eot")
                with nc.allow_low_precision(reason="bf16 output, host upcasts"):
                    nc.scalar.activation(otp, banks[b], IDENT, bias=w["eb3"])
                e0a = grp[2 * b][0] * TN
                e0b = grp[2 * b + 1][0] * TN
                nc.sync.dma_start(outT_d[:, e0a:e0a + TN], otp[0:64])
                nc.sync.dma_start(outT_d[:, e0b:e0b + TN], otp[64:128])
            pend = pend[nfl:]
        st["prev2"] = st["prev"]
        st["prev"] = cur
        st["pend"] = pend


def _emit_edge_stage(nc, pools, w, attrT_d, outT_d, fe_src):
    st = {"prev": None, "prev2": None, "pend": [], "rtpq": {}, "et": ET}
    for t in range(ET):
        _edge_step(nc, pools, w, attrT_d, outT_d, fe_src, st, t)
    for _ in range(4):
        _edge_step(nc, pools, w, attrT_d, outT_d, fe_src, st, None)


def _build(mode):
    """mode: 'fused' (node+edge, fe on-chip), 'node', 'edge'."""
    nc = bacc.Bacc("TRN2", target_bir_lowering=False)
    td = _declare_weights(nc)
    if mode in ("fused", "node"):
        xT_d = nc.dram_tensor("xT", [128, RC], BF16, kind="ExternalInput")
    if mode in ("fused", "edge"):
        attrT_d = nc.dram_tensor("attrT", [64, EC], BF16, kind="ExternalInput")
        outT_d = nc.dram_tensor("outT", [64, EC], BF16, kind="ExternalOutput")
    if mode == "edge":
        feTg_d = nc.dram_tensor("feTg", [64, EC], BF16, kind="ExternalInput")
    if mode == "node":
        feT_out = nc.dram_tensor("feT", [ODE, GC], F32, kind="ExternalOutput")

    with tile.TileContext(nc) as tc:
        with (
            tc.tile_pool(name="consts", bufs=1) as consts,
            tc.tile_pool(name="xin", bufs=10) as xin,
            tc.tile_pool(name="hid", bufs=8) as hid,
            tc.tile_pool(name="ps1", bufs=1, space="PSUM") as ps1,
            tc.tile_pool(name="ps2", bufs=2, space="PSUM") as ps2,
            tc.tile_pool(name="ps3", bufs=2, space="PSUM") as ps3,
        ):
            pools = (consts, xin, hid, ps1, ps2, ps3)
            w = _load_weights(nc, consts, td,
                              node=mode in ("fused", "node"),
                              edge=mode == "edge")
            if mode == "fused":
                # unified pipeline: node tile t front || node back(t-2) ||
                # feature_enc batch of 4 chunks || the 4 edge tiles of chunk t-5.
                hsum = consts.tile([128, 2, NT, 16], BF16, tag="hsum")
                feT_sb = consts.tile([ODE, GC], BF16, tag="feT")
                fe4T = consts.tile([ODE, GC, 4], BF16, tag="fe4T")
                fe_src = ("sbuf", fe4T)
                st = {"prev": None, "prev2": None, "pend": [], "rtpq": {}, "et": ET}
                prev = prev2 = None
                pending_batch = None
                for t in range(NT + 6):
                    cur = _node_front(nc, pools, w, xT_d, t) if t < NT else None
                    if t == 0:
                        _load_weights(nc, consts, td, node=False, edge=True, sb=w)
                    if pending_batch is not None:
                        _fe_chunk(nc, pools, w, hsum, feT_sb, fe4T, pending_batch, 4)
                        pending_batch = None
                    if prev2 is not None:
                        _node_back(nc, pools, w, hsum, prev2)
                        if prev2[0] % 4 == 3:
                            pending_batch = prev2[0] - 3
                    c = t - 6
                    if c >= 0:
                        for e in range(4 * c, 4 * c + 4):
                            _edge_step(nc, pools, w, attrT_d, outT_d, fe_src, st, e)
                    prev2 = prev
                    prev = cur
                for _ in range(4):
                    _edge_step(nc, pools, w, attrT_d, outT_d, fe_src, st, None)
            elif mode == "node":
                hsum = consts.tile([128, 2, NT, 16], BF16, tag="hsum")
                prev = prev2 = None
                for t in range(NT + 2):
                    cur = _node_front(nc, pools, w, xT_d, t) if t < NT else None
                    if prev2 is not None:
                        _node_back(nc, pools, w, hsum, prev2)
                    prev2 = prev
                    prev = cur
                ps_f = ps3.tile([128, TN], F32, tag="l3")
                hs = hsum.rearrange("c k t g -> c k (t g)")
                for k in (0, 1):
                    nc.tensor.matmul(ps_f[0:64], w["nw3"][:, k], hs[:, k],
                                     start=(k == 0), stop=(k == 1), tile_position=(0, 0))
                feTf = consts.tile([ODE, GC], F32, tag="feTf")
                nc.scalar.activation(feTf, ps_f[0:64], IDENT, bias=w["nb3"])
                nc.sync.dma_start(feT_out[:], feTf)
            elif mode == "edge":
                _emit_edge_stage(nc, pools, w, attrT_d, outT_d, ("dram", feTg_d))
    nc.finalize()
    return nc


def _get_program(mode):
    if mode not in _PROGRAMS:
        _PROGRAMS[mode] = _build(mode)
    return _PROGRAMS[mode]


def _shared_weight_arrays(kw):
    f = np.float32
    c = np.ascontiguousarray
    b3 = np.asarray(kw["edge_b3"], dtype=f)
    return {
        "nw1": c(np.asarray(kw["node_w1"], dtype=f).astype(BF16NP)),
        "nw2": c(np.asarray(kw["node_w2"], dtype=f).reshape(2, 128, H).transpose(1, 0, 2).astype(BF16NP)),
        "nw3": c(np.asarray(kw["node_w3"], dtype=f).reshape(2, 128, ODE).transpose(1, 0, 2).astype(BF16NP)),
        "nb1": c(np.asarray(kw["node_b1"], dtype=f).reshape(2, 128).T),
        "nb2": c(np.asarray(kw["node_b2"], dtype=f).reshape(2, 128).T),
        "nb3": c(np.asarray(kw["node_b3"], dtype=f).reshape(ODE, 1)),
        "ew1": c(np.asarray(kw["edge_w1"], dtype=f).astype(BF16NP)),
        "ew2": c(np.asarray(kw["edge_w2"], dtype=f).reshape(2, 128, H).transpose(1, 0, 2).astype(BF16NP)),
        "ew3": c(np.asarray(kw["edge_w3"], dtype=f).reshape(2, 128, ODE).transpose(1, 0, 2).astype(BF16NP)),
        "eb1": c(np.asarray(kw["edge_b1"], dtype=f).reshape(2, 128).T),
        "eb2": c(np.asarray(kw["edge_b2"], dtype=f).reshape(2, 128).T),
        "eb3": c(np.concatenate([b3, b3]).reshape(128, 1)),
    }


def _x_transposed_per_core(x, c):
    xs = np.asarray(x, dtype=np.float32).reshape(G, ODE, 2, NDATA)[c * GC:(c + 1) * GC]
    return np.ascontiguousarray(xs.transpose(1, 2, 0, 3).reshape(128, RC).astype(BF16NP))


def kernel(x, edge_attr, node_w1, node_b1, node_w2, node_b2, node_w3, node_b3,
           edge_w1, edge_b1, edge_w2, edge_b2, edge_w3, edge_b3,
           edge_index, batch):
    global last_results
    kw = dict(x=x, node_w1=node_w1, node_b1=node_b1, node_w2=node_w2,
              node_b2=node_b2, node_w3=node_w3, node_b3=node_b3,
              edge_w1=edge_w1, edge_b1=edge_b1, edge_w2=edge_w2,
              edge_b2=edge_b2, edge_w3=edge_w3, edge_b3=edge_b3)
    trace = os.environ.get("KERNEL_TRACE", "") == "1"
    if trace:
        _install_trace_shim()

    edge_attr = np.asarray(edge_attr, dtype=np.float32)
    ei = np.asarray(edge_index)
    bt = np.asarray(batch)
    g_src = bt[ei[0]]
    g_dst = bt[ei[1]]
    same = g_src == g_dst
    structured = bool((g_src == np.repeat(np.arange(G), EPG)).all())

    shared = _shared_weight_arrays(kw)
    run_kwargs = dict(core_ids=list(range(NCORES)), trace=trace,
                      trace_cores=[0] if trace else None)

    if structured:
        nc = _get_program("fused")
        in_maps = []
        for c in range(NCORES):
            m = dict(shared)
            m["xT"] = _x_transposed_per_core(x, c)
            m["attrT"] = np.ascontiguousarray(edge_attr[c * EC:(c + 1) * EC].T.astype(BF16NP))
            in_maps.append(m)
        res = run_bass_kernel_spmd(nc, in_maps, **run_kwargs)
        last_results = res
        out = np.empty((E, EA), dtype=np.float32)
        for c in range(NCORES):
            out[c * EC:(c + 1) * EC] = res.results[c]["outT"].T.astype(np.float32)
    else:
        # general path: node stage -> host gather of feature_enc -> edge stage
        nc_node = _get_program("node")
        in_maps = []
        for c in range(NCORES):
            m = dict(shared)
            m["xT"] = _x_transposed_per_core(x, c)
            in_maps.append(m)
        res_n = run_bass_kernel_spmd(nc_node, in_maps, **run_kwargs)
        feT_full = np.concatenate([res_n.results[c]["feT"] for c in range(NCORES)],
                                  axis=1)          # [64, G]
        feTg = feT_full[:, g_src]                   # [64, E]
        nc_edge = _get_program("edge")
        in_maps = []
        for c in range(NCORES):
            m = dict(shared)
            m["attrT"] = np.ascontiguousarray(edge_attr[c * EC:(c + 1) * EC].T.astype(BF16NP))
            m["feTg"] = np.ascontiguousarray(feTg[:, c * EC:(c + 1) * EC].astype(BF16NP))
            in_maps.append(m)
        res = run_bass_kernel_spmd(nc_edge, in_maps, **run_kwargs)
        last_results = res
        out = np.empty((E, EA), dtype=np.float32)
        for c in range(NCORES):
            out[c * EC:(c + 1) * EC] = res.results[c]["outT"].T.astype(np.float32)
    if not same.all():
        out = np.where(same[:, None], out, edge_attr)
    return out


# revision 27
# speedup vs baseline: 1.0145x; 1.0043x over previous
"""Trainium2 Bass kernel for nn_NodeNet (GNN message passing).

Strategy: data-parallel over graphs across 8 NeuronCores. Host transposes
inputs into [feature, row] layouts so every DMA is contiguous; all matmuls
bf16 (fp8 DoubleRow was tested and rejected: quantizing h1 to e4m3 alone
costs 2.6e-2 max-rel error, over the 2e-2 budget).

One fused launch per core runs a single unified software pipeline:
  - node tile t front (DMA x, l1, bias/relu) alongside node back(t-2)
    (l2, relu, datapoint-sum as 2x-mode tensor_tensor halving adds + reduce),
  - feature_enc emitted in batches of 4 node tiles (one N=64 matmul pair),
    delayed one iteration so the PE never waits on the DVE reduce,
  - the 4 edge tiles of node-chunk t-6, themselves 2-deep pipelined
    (front: attr DMA + fe broadcast from a x4-duplicated table + l1 + h1;
    back at t-2: l2 + h2; l3 for pairs of tiles is flushed one step late and
    packed into a single [128,512] PSUM bank via column tile_position, which
    halves the output-bias pointwise work and lets the two M=64 groups run
    concurrently in the PE array).
The next tile's attr DMA + fe broadcast are prefetched ahead of the current
pointwise ops so edge l1 never waits on the Vector queue. PSUM->SBUF pointwise
is split Scalar/Vector per the errata cost model ((172+FD)/1.2 vs
(120+FD)/0.96); output is written bf16 and upcast on the host. PSUM budget:
l1 a+b (2 banks) + l2 a+b double-buffered (4) + l3/feature_enc ring (2) = 8.

Structured fast path (edges grouped 128-per-graph, as produced by the
reference's setup_inputs) uses the fused program; a general fallback handles
arbitrary edge_index / batch via a node launch, host gather, edge launch.
Measured: 293.4us HW exec (baseline 425.6us), max rel err 7.5e-3.
"""

import os
import sys

import ml_dtypes
import numpy as np

BF16NP = ml_dtypes.bfloat16

if "/opt/trn_rl_repo" not in sys.path and os.path.isdir("/opt/trn_rl_repo"):
    sys.path.insert(0, "/opt/trn_rl_repo")

import concourse.bacc as bacc
import concourse.tile as tile
from concourse import mybir
from concourse.bass_utils import run_bass_kernel_spmd

G, ODE, NDATA, H, EA, EPG = 4096, 64, 32, 256, 64, 128
E = G * EPG
NCORES = 8
GC = G // NCORES           # graphs per core (512)
RC = GC * NDATA            # node-MLP rows per core (16384)
EC = GC * EPG              # edges per core (65536)
TN = 512                   # tile free size
NT = RC // TN              # node tiles (32)
ET = EC // TN              # edge tiles (128)
GPT = TN // EPG            # graphs per edge tile (4)

F32 = mybir.dt.float32
BF16 = mybir.dt.bfloat16
RELU = mybir.ActivationFunctionType.Relu
IDENT = mybir.ActivationFunctionType.Identity
ADD = mybir.AluOpType.add
MAX = mybir.AluOpType.max

_PROGRAMS = {}
last_results = None


def _install_trace_shim():
    """Optional: make trace=True work by injecting antenv.axon_hooks."""
    import types

    if "antenv.axon_hooks" in sys.modules:
        return
    try:
        mod = types.ModuleType("antenv.axon_hooks")
        mod._hook = None
        mod.set_axon_ntff_profile_hook = lambda h: setattr(mod, "_hook", h)
        mod.get_axon_ntff_profile_hook = lambda: mod._hook
        sys.modules["antenv.axon_hooks"] = mod
        import antenv

        antenv.axon_hooks = mod
        from trn_agent_boot.trn_boot import _ntff_profile_via_ctypes

        hook = _ntff_profile_via_ctypes("/opt/axon/libaxon_pjrt.so")
        if hook is not None:
            mod.set_axon_ntff_profile_hook(hook)
    except Exception:
        pass


def _declare_weights(nc):
    # all bf16 weights packed column-wise into one blob, biases into another:
    # wpk = [nw1(256) | nw2(512) | nw3(128) | ew1(256) | ew2(512) | ew3(128)]
    # bpk = [nb1(2) | nb2(2) | eb1(2) | eb2(2) | eb3(1)]
    t = {}
    t["wpk"] = nc.dram_tensor("wpk", [128, 1792], BF16, kind="ExternalInput")
    t["bpk"] = nc.dram_tensor("bpk", [128, 9], F32, kind="ExternalInput")
    t["nb3"] = nc.dram_tensor("nb3", [ODE, 1], F32, kind="ExternalInput")
    return t


def _load_weights(nc, consts, td, node: bool, edge: bool, sb=None):
    wpk = consts.tile([128, 1792], BF16, tag="wpk", name="wpk")
    bpk = consts.tile([128, 9], F32, tag="bpk", name="bpk")
    nb3 = consts.tile([ODE, 1], F32, tag="nb3", name="nb3")
    nc.sync.dma_start(wpk, td["wpk"][:])
    nc.sync.dma_start(bpk, td["bpk"][:])
    nc.sync.dma_start(nb3, td["nb3"][:])
    sb = {
        "nw1": wpk[:, 0:256],
        "nw2": wpk[:, 256:768].rearrange("c (k h) -> c k h", k=2),
        "nw3": wpk[:, 768:896].rearrange("c (k h) -> c k h", k=2),
        "ew1": wpk[:, 896:1152],
        "ew2": wpk[:, 1152:1664].rearrange("c (k h) -> c k h", k=2),
        "ew3": wpk[:, 1664:1792].rearrange("c (k h) -> c k h", k=2),
        "nb1": bpk[:, 0:2],
        "nb2": bpk[:, 2:4],
        "eb1": bpk[:, 4:6],
        "eb2": bpk[:, 6:8],
        "eb3": bpk[:, 8:9],
        "nb3": nb3,
    }
    return sb


def _node_front(nc, pools, w, xT_d, t):
    consts, xin, hid, ps1, ps2, ps3 = pools
    xtp = xin.tile([128, TN], BF16, tag="nxt")
    nc.sync.dma_start(xtp, xT_d[:, t * TN:(t + 1) * TN])
    pa = ps1.tile([128, TN], F32, tag="l1a")
    pb = ps1.tile([128, TN], F32, tag="l1b")
    nc.tensor.matmul(pa, w["nw1"][:, 0:128], xtp, start=True, stop=True)
    nc.tensor.matmul(pb, w["nw1"][:, 128:256], xtp, start=True, stop=True)
    h1p = hid.tile([128, 2, TN], BF16, tag="nh1")
    nc.scalar.activation(h1p[:, 0], pa, RELU, bias=w["nb1"][:, 0:1])
    nc.vector.tensor_scalar(out=h1p[:, 1], in0=pb, scalar1=w["nb1"][:, 1:2],
                            scalar2=0.0, op0=ADD, op1=MAX)
    return (t, h1p)


def _node_back(nc, pools, w, hsum, prev2):
    consts, xin, hid, ps1, ps2, ps3 = pools
    tp, h1p_p = prev2
    pa = ps2.tile([128, TN], F32, tag="l2a")
    pb = ps2.tile([128, TN], F32, tag="l2b")
    for k in (0, 1):
        nc.tensor.matmul(pa, w["nw2"][:, k, 0:128], h1p_p[:, k],
                         start=(k == 0), stop=(k == 1))
    for k in (0, 1):
        nc.tensor.matmul(pb, w["nw2"][:, k, 128:256], h1p_p[:, k],
                         start=(k == 0), stop=(k == 1))
    h2p = hid.tile([128, 2, TN], BF16, tag="nh2")
    nc.scalar.activation(h2p[:, 0], pa, RELU, bias=w["nb2"][:, 0:1])
    nc.scalar.activation(h2p[:, 1], pb, RELU, bias=w["nb2"][:, 1:2])
    # sum over NDATA=32 datapoints of each of the 16 graphs:
    # two 2x-mode halving adds then one 1x reduce over the last 8
    with nc.allow_low_precision(reason="bf16 tree reduce feeds bf16 matmul"):
        v0 = h2p.rearrange("c k (g d) -> c k g d", d=NDATA)
        t1 = hid.tile([128, 2, 16, 16], BF16, tag="tr1")
        nc.vector.tensor_tensor(out=t1, in0=v0[:, :, :, 0:16], in1=v0[:, :, :, 16:32], op=ADD)
        t2 = hid.tile([128, 2, 16, 8], BF16, tag="tr2")
        nc.vector.tensor_tensor(out=t2, in0=t1[:, :, :, 0:8], in1=t1[:, :, :, 8:16], op=ADD)
        nc.vector.reduce_sum(out=hsum[:, :, tp], in_=t2,
                             axis=mybir.AxisListType.X)


def _fe_chunk(nc, pools, w, hsum, feT_sb, fe4T, c0, nch):
    """feature_enc for node tiles c0..c0+nch-1 (16 graphs each)."""
    consts, xin, hid, ps1, ps2, ps3 = pools
    ng = nch * 16
    ps_f = ps3.tile([128, TN], F32, tag="l3")
    for k in (0, 1):
        nc.tensor.matmul(ps_f[0:64, 0:ng], w["nw3"][:, k], hsum[:, k, c0:c0 + nch],
                         start=(k == 0), stop=(k == 1), tile_position=(0, 0))
    sl = slice(c0 * 16, c0 * 16 + ng)
    nc.scalar.activation(feT_sb[:, sl], ps_f[0:64, 0:ng], IDENT, bias=w["nb3"])
    with nc.allow_low_precision(reason="bf16 broadcast of bf16 data"):
        nc.vector.tensor_copy(
            out=fe4T[:, sl], in_=feT_sb[:, sl, None].to_broadcast([ODE, ng, 4]))


def _edge_prefetch(nc, pools, w, attrT_d, fe_src, st, t):
    consts, xin, hid, ps1, ps2, ps3 = pools
    e0 = t * TN
    g0 = t * GPT
    rtp = xin.tile([128, TN], BF16, tag="ert")
    nc.sync.dma_start(rtp[64:128], attrT_d[:, e0:e0 + TN])
    if fe_src[0] == "sbuf":
        fe4T = fe_src[1]
        # rtp[0:64] cols: e = glocal*128 + r*4 + f
        nc.vector.tensor_copy(
            out=rtp[0:64].rearrange("c (g r f) -> c g r f", g=GPT, f=4),
            in_=fe4T[:, g0:g0 + GPT, None, :].to_broadcast([ODE, GPT, EPG // 4, 4]),
        )
    else:
        nc.sync.dma_start(rtp[0:64], fe_src[1][:, e0:e0 + TN])
    st["rtpq"][t] = rtp


def _edge_step(nc, pools, w, attrT_d, outT_d, fe_src, st, t):
    """One pipelined edge step, 2 deep: front(t) = dma+bcast+l1+h1; back of
    st.prev2 (tile t-2) = l2+h2; l3/e3/out for the oldest pending pair once a
    third tile is pending, packed into one [128,TN] PSUM bank via column
    tile_position. st = dict(prev=..., prev2=..., pend=[...]). t=None -> drain."""
    consts, xin, hid, ps1, ps2, ps3 = pools
    prev = st["prev2"]
    pend = st["pend"]
    if True:
        cur = None
        if t is not None:
            if t not in st["rtpq"]:
                _edge_prefetch(nc, pools, w, attrT_d, fe_src, st, t)
            rtp = st["rtpq"].pop(t)
            pa = ps1.tile([128, TN], F32, tag="l1a")
            pb = ps1.tile([128, TN], F32, tag="l1b")
            nc.tensor.matmul(pa, w["ew1"][:, 0:128], rtp, start=True, stop=True)
            nc.tensor.matmul(pb, w["ew1"][:, 128:256], rtp, start=True, stop=True)
            # prefetch the next tile's inputs so its l1 never waits on the
            # broadcast; across a fe-batch boundary the columns aren't written
            # yet, so skip (that tile prefetches at its own step).
            if t + 1 < st["et"] and (t + 1) % 16 != 0:
                _edge_prefetch(nc, pools, w, attrT_d, fe_src, st, t + 1)
            h1p = hid.tile([128, 2, TN], BF16, tag="eh1")
            nc.scalar.activation(h1p[:, 0], pa, RELU, bias=w["eb1"][:, 0:1])
            nc.vector.tensor_scalar(out=h1p[:, 1], in0=pb, scalar1=w["eb1"][:, 1:2],
                                    scalar2=0.0, op0=ADD, op1=MAX)
            cur = (t, h1p)
        if prev is not None:
            tp, h1p_p = prev
            pa = ps2.tile([128, TN], F32, tag="l2a")
            pb = ps2.tile([128, TN], F32, tag="l2b")
            for k in (0, 1):
                nc.tensor.matmul(pa, w["ew2"][:, k, 0:128], h1p_p[:, k],
                                 start=(k == 0), stop=(k == 1))
            for k in (0, 1):
                nc.tensor.matmul(pb, w["ew2"][:, k, 128:256], h1p_p[:, k],
                                 start=(k == 0), stop=(k == 1))
            h2p = hid.tile([128, 2, TN], BF16, tag="eh2")
            nc.scalar.activation(h2p[:, 0], pa, RELU, bias=w["eb2"][:, 0:1])
            nc.vector.tensor_scalar(out=h2p[:, 1], in0=pb, scalar1=w["eb2"][:, 1:2],
                                    scalar2=0.0, op0=ADD, op1=MAX)
            pend.append((tp, h2p))
        nfl = 2 if len(pend) >= 3 else (len(pend) // 2 * 2 if t is None else 0)
        if nfl:
            # l3 for nfl pending tiles, col-tiled two-per-PSUM-bank: within a
            # bank the first tile -> partitions 0-63, second -> 64-127.
            # One k-outer burst across all tiles so the column groups run
            # concurrently and LDWEIGHTS stays covered.
            grp = pend[:nfl]
            banks = [ps3.tile([128, TN], F32, tag="l3", name=f"l3b{i}")
                     for i in range(nfl // 2)]
            for k in (0, 1):
                for j, (tj, h2j) in enumerate(grp):
                    nc.tensor.matmul(banks[j // 2][64 * (j % 2):64 * (j % 2) + 64],
                                     w["ew3"][:, k], h2j[:, k],
                                     start=(k == 0), stop=(k == 1),
                                     tile_position=(0, 64 * (j % 2)))
            for b in range(nfl // 2):
                otp = hid.tile([128, TN], BF16, tag="eot")
                with nc.allow_low_precision(reason="bf16 output, host upcasts"):
                    nc.scalar.activation(otp, banks[b], IDENT, bias=w["eb3"])
                e0a = grp[2 * b][0] * TN
                e0b = grp[2 * b + 1][0] * TN
                nc.sync.dma_start(outT_d[:, e0a:e0a + TN], otp[0:64])
                nc.sync.dma_start(outT_d[:, e0b:e0b + TN], otp[64:128])
            pend = pend[nfl:]
        st["prev2"] = st["prev"]
        st["prev"] = cur
        st["pend"] = pend


def _emit_edge_stage(nc, pools, w, attrT_d, outT_d, fe_src):
    st = {"prev": None, "prev2": None, "pend": [], "rtpq": {}, "et": ET}
    for t in range(ET):
        _edge_step(nc, pools, w, attrT_d, outT_d, fe_src, st, t)
    for _ in range(4):
        _edge_step(nc, pools, w, attrT_d, outT_d, fe_src, st, None)


def _build(mode):
    """mode: 'fused' (node+edge, fe on-chip), 'node', 'edge'."""
    nc = bacc.Bacc("TRN2", target_bir_lowering=False)
    td = _declare_weights(nc)
    if mode in ("fused", "node"):
        xT_d = nc.dram_tensor("xT", [128, RC], BF16, kind="ExternalInput")
    if mode in ("fused", "edge"):
        attrT_d = nc.dram_tensor("attrT", [64, EC], BF16, kind="ExternalInput")
        outT_d = nc.dram_tensor("outT", [64, EC], BF16, kind="ExternalOutput")
    if mode == "edge":
        feTg_d = nc.dram_tensor("feTg", [64, EC], BF16, kind="ExternalInput")
    if mode == "node":
        feT_out = nc.dram_tensor("feT", [ODE, GC], F32, kind="ExternalOutput")

    with tile.TileContext(nc) as tc:
        with (
            tc.tile_pool(name="consts", bufs=1) as consts,
            tc.tile_pool(name="xin", bufs=10) as xin,
            tc.tile_pool(name="hid", bufs=8) as hid,
            tc.tile_pool(name="ps1", bufs=1, space="PSUM") as ps1,
            tc.tile_pool(name="ps2", bufs=2, space="PSUM") as ps2,
            tc.tile_pool(name="ps3", bufs=2, space="PSUM") as ps3,
        ):
            pools = (consts, xin, hid, ps1, ps2, ps3)
            w = _load_weights(nc, consts, td,
                              node=mode in ("fused", "node"),
                              edge=mode == "edge")
            if mode == "fused":
                # unified pipeline: node tile t front || node back(t-2) ||
                # feature_enc batch of 4 chunks || the 4 edge tiles of chunk t-5.
                hsum = consts.tile([128, 2, NT, 16], BF16, tag="hsum")
                feT_sb = consts.tile([ODE, GC], BF16, tag="feT")
                fe4T = consts.tile([ODE, GC, 4], BF16, tag="fe4T")
                fe_src = ("sbuf", fe4T)
                st = {"prev": None, "prev2": None, "pend": [], "rtpq": {}, "et": ET}
                prev = prev2 = None
                pending_batch = None
                for t in range(NT + 6):
                    cur = _node_front(nc, pools, w, xT_d, t) if t < NT else None
                    if pending_batch is not None:
                        _fe_chunk(nc, pools, w, hsum, feT_sb, fe4T, pending_batch, 4)
                        pending_batch = None
                    if prev2 is not None:
                        _node_back(nc, pools, w, hsum, prev2)
                        if prev2[0] % 4 == 3:
                            pending_batch = prev2[0] - 3
                    c = t - 6
                    if c >= 0:
                        for e in range(4 * c, 4 * c + 4):
                            _edge_step(nc, pools, w, attrT_d, outT_d, fe_src, st, e)
                    prev2 = prev
                    prev = cur
                for _ in range(4):
                    _edge_step(nc, pools, w, attrT_d, outT_d, fe_src, st, None)
            elif mode == "node":
                hsum = consts.tile([128, 2, NT, 16], BF16, tag="hsum")
                prev = prev2 = None
                for t in range(NT + 2):
                    cur = _node_front(nc, pools, w, xT_d, t) if t < NT else None
                    if prev2 is not None:
                        _node_back(nc, pools, w, hsum, prev2)
                    prev2 = prev
                    prev = cur
                ps_f = ps3.tile([128, TN], F32, tag="l3")
                hs = hsum.rearrange("c k t g -> c k (t g)")
                for k in (0, 1):
                    nc.tensor.matmul(ps_f[0:64], w["nw3"][:, k], hs[:, k],
                                     start=(k == 0), stop=(k == 1), tile_position=(0, 0))
                feTf = consts.tile([ODE, GC], F32, tag="feTf")
                nc.scalar.activation(feTf, ps_f[0:64], IDENT, bias=w["nb3"])
                nc.sync.dma_start(feT_out[:], feTf)
            elif mode == "edge":
                _emit_edge_stage(nc, pools, w, attrT_d, outT_d, ("dram", feTg_d))
    nc.finalize()
    return nc


def _get_program(mode):
    if mode not in _PROGRAMS:
        _PROGRAMS[mode] = _build(mode)
    return _PROGRAMS[mode]


def _shared_weight_arrays(kw):
    f = np.float32
    c = np.ascontiguousarray
    b3 = np.asarray(kw["edge_b3"], dtype=f)
    wpk = np.concatenate([
        np.asarray(kw["node_w1"], dtype=f),
        np.asarray(kw["node_w2"], dtype=f).reshape(2, 128, H).transpose(1, 0, 2).reshape(128, 2 * H),
        np.asarray(kw["node_w3"], dtype=f).reshape(2, 128, ODE).transpose(1, 0, 2).reshape(128, 2 * ODE),
        np.asarray(kw["edge_w1"], dtype=f),
        np.asarray(kw["edge_w2"], dtype=f).reshape(2, 128, H).transpose(1, 0, 2).reshape(128, 2 * H),
        np.asarray(kw["edge_w3"], dtype=f).reshape(2, 128, ODE).transpose(1, 0, 2).reshape(128, 2 * ODE),
    ], axis=1)
    bpk = np.concatenate([
        np.asarray(kw["node_b1"], dtype=f).reshape(2, 128).T,
        np.asarray(kw["node_b2"], dtype=f).reshape(2, 128).T,
        np.asarray(kw["edge_b1"], dtype=f).reshape(2, 128).T,
        np.asarray(kw["edge_b2"], dtype=f).reshape(2, 128).T,
        np.concatenate([b3, b3]).reshape(128, 1),
    ], axis=1)
    return {
        "wpk": c(wpk.astype(BF16NP)),
        "bpk": c(bpk),
        "nb3": c(np.asarray(kw["node_b3"], dtype=f).reshape(ODE, 1)),
    }


def _x_transposed_per_core(x, c):
    xs = np.asarray(x, dtype=np.float32).reshape(G, ODE, 2, NDATA)[c * GC:(c + 1) * GC]
    return np.ascontiguousarray(xs.transpose(1, 2, 0, 3).reshape(128, RC).astype(BF16NP))


def kernel(x, edge_attr, node_w1, node_b1, node_w2, node_b2, node_w3, node_b3,
           edge_w1, edge_b1, edge_w2, edge_b2, edge_w3, edge_b3,
           edge_index, batch):
    global last_results
    kw = dict(x=x, node_w1=node_w1, node_b1=node_b1, node_w2=node_w2,
              node_b2=node_b2, node_w3=node_w3, node_b3=node_b3,
              edge_w1=edge_w1, edge_b1=edge_b1, edge_w2=edge_w2,
              edge_b2=edge_b2, edge_w3=edge_w3, edge_b3=edge_b3)
    trace = os.environ.get("KERNEL_TRACE", "") == "1"
    if trace:
        _install_trace_shim()

    edge_attr = np.asarray(edge_attr, dtype=np.float32)
    ei = np.asarray(edge_index)
    bt = np.asarray(batch)
    g_src = bt[ei[0]]
    g_dst = bt[ei[1]]
    same = g_src == g_dst
    structured = bool((g_src == np.repeat(np.arange(G), EPG)).all())

    shared = _shared_weight_arrays(kw)
    run_kwargs = dict(core_ids=list(range(NCORES)), trace=trace,
                      trace_cores=[0] if trace else None)

    if structured:
        nc = _get_program("fused")
        in_maps = []
        for c in range(NCORES):
            m = dict(shared)
            m["xT"] = _x_transposed_per_core(x, c)
            m["attrT"] = np.ascontiguousarray(edge_attr[c * EC:(c + 1) * EC].T.astype(BF16NP))
            in_maps.append(m)
        res = run_bass_kernel_spmd(nc, in_maps, **run_kwargs)
        last_results = res
        out = np.empty((E, EA), dtype=np.float32)
        for c in range(NCORES):
            out[c * EC:(c + 1) * EC] = res.results[c]["outT"].T.astype(np.float32)
    else:
        # general path: node stage -> host gather of feature_enc -> edge stage
        nc_node = _get_program("node")
        in_maps = []
        for c in range(NCORES):
            m = dict(shared)
            m["xT"] = _x_transposed_per_core(x, c)
            in_maps.append(m)
        res_n = run_bass_kernel_spmd(nc_node, in_maps, **run_kwargs)
        feT_full = np.concatenate([res_n.results[c]["feT"] for c in range(NCORES)],
                                  axis=1)          # [64, G]
        feTg = feT_full[:, g_src]                   # [64, E]
        nc_edge = _get_program("edge")
        in_maps = []
        for c in range(NCORES):
            m = dict(shared)
            m["attrT"] = np.ascontiguousarray(edge_attr[c * EC:(c + 1) * EC].T.astype(BF16NP))
            m["feTg"] = np.ascontiguousarray(feTg[:, c * EC:(c + 1) * EC].astype(BF16NP))
            in_maps.append(m)
        res = run_bass_kernel_spmd(nc_edge, in_maps, **run_kwargs)
        last_results = res
        out = np.empty((E, EA), dtype=np.float32)
        for c in range(NCORES):
            out[c * EC:(c + 1) * EC] = res.results[c]["outT"].T.astype(np.float32)
    if not same.all():
        out = np.where(same[:, None], out, edge_attr)
    return out
